# revision 22
# baseline (speedup 1.0000x reference)
"""Trainium2 Bass kernel for nn_ClassificationHead: LayerNorm -> Linear(1024,256) -> GELU -> Linear(256,2).

Data-parallel over 8 NeuronCores: each core processes 8192 rows of the
65536-row batch; the tiny weights are replicated. The host supplies each
core's shard pre-transposed in bf16 (layout-only prep: [1024, 8192],
K-major as the tensor engine requires); all math runs on device.

Per-core pipeline, per 512-row block (4 tiles of 128 rows):
  1. One DMA loads the K-major block [128, 8, 512] bf16.
  2. Per tile, TensorE runs 8 accumulating matmuls against W1aug
     ([W1' | ones] -> PSUM cols 0:256 = x @ W1', col 256 = rowsum), plus a
     Gram matmul reusing the already-loaded stationary x-chunk
     (ldweights=False) into PSUM cols 257:385.
  3. DVE extracts -mu (from the rowsum col) and sum(x^2) (Gram diagonal via
     tensor_tensor_reduce against an identity); a batched Newton-rsqrt
     (bit-trick seed + 2 iterations) gives g = 1/sqrt(var+eps) and
     rhat = 1/g. A tiny [128,128] xbar-transpose DMA flips the per-row
     stats into rows.
  4. TensorE adds the rank-2 correction (-mu ox s1 + rhat ox c1), so after
     the GELU's per-partition scale g the PSUM holds exactly LN(x)@W1'+b1.
  5. ACT evaluates exact GELU with scale g -> bf16 h tile.
  6. TensorE transposes h (via identity), ACT evacuates PSUM->SBUF bf16,
     TensorE computes h @ W2; DVE adds b2 into a staging tile.
  7. One DMA writes the [8192, 2] fp32 result back.

Host-side weight folding (tiny, O(1MB)): W1' = ln_w[:,None]*W1,
s1 = colsum(W1'), c1 = ln_b@W1 + b1.
"""
import sys

sys.path.insert(0, "/opt/trn_rl_repo")
sys.path.insert(0, "/root/.axon_site")

import numpy as np
import ml_dtypes

N_CORES = 8
BATCH = 65536
D = 1024
H = 256
OUT = 2
RPC = BATCH // N_CORES  # rows per core
NT = RPC // 128         # 128-row tiles per core
KC = D // 128           # contraction chunks
G = 4                   # tiles per block (512 rows)
NB = NT // G            # blocks per core
EPS = 1e-5
MAGIC = 0x5F3759DF

_cache = {}


def _bf16(a):
    return np.asarray(a, dtype=ml_dtypes.bfloat16)


def _build(rpc=RPC):
    import concourse.bacc as bacc
    from concourse.tile_rust import add_dep_helper
    import concourse.mybir as mybir
    from concourse import tile

    f32 = mybir.dt.float32
    i32 = mybir.dt.int32
    bf16 = mybir.dt.bfloat16
    AF = mybir.ActivationFunctionType
    ALU = mybir.AluOpType

    nc = bacc.Bacc(None, target_bir_lowering=False, debug=False)

    xt_in = nc.dram_tensor("xt", [D, rpc], bf16, kind="ExternalInput")
    w1_in = nc.dram_tensor("w1aug", [128, KC, H + 1], bf16, kind="ExternalInput")
    sc_in = nc.dram_tensor("screp", [2 * G, G, H + 1], bf16, kind="ExternalInput")
    w2_in = nc.dram_tensor("w2rep", [128, OUT, H], bf16, kind="ExternalInput")
    b2_in = nc.dram_tensor("b2g", [128, G * OUT], f32, kind="ExternalInput")
    idf_in = nc.dram_tensor("identf", [128, 128], f32, kind="ExternalInput")
    y_out = nc.dram_tensor("y", [rpc, OUT], f32, kind="ExternalOutput")
    y_v = y_out.rearrange("(t p) c -> p t c", p=128)

    xt_v = xt_in.rearrange("(c p) r -> p c r", p=128)   # [128, KC, RPC]

    with tile.TileContext(nc) as tc:
        with (
            tc.tile_pool(name="wpool", bufs=1) as wp,
            tc.tile_pool(name="xtp", bufs=4) as xtp,
            tc.tile_pool(name="statp", bufs=2) as statp,
            tc.tile_pool(name="scrp", bufs=2) as scrp,
            tc.tile_pool(name="hbp", bufs=3) as hbp,
            tc.tile_pool(name="htp", bufs=3) as htp,
            tc.tile_pool(name="outp", bufs=1) as outp,
            tc.tile_pool(name="pszp", bufs=7, space="PSUM") as pszp,
            tc.tile_pool(name="psgp", bufs=1, space="PSUM") as psgp,
        ):
            w1sb = wp.tile([128, KC, H + 1], bf16)
            nc.scalar.dma_start(w1sb[:], w1_in[:])
            scsb = wp.tile([2 * G, G, H + 1], bf16)
            nc.scalar.dma_start(scsb[:], sc_in[:])
            w2sb = wp.tile([128, OUT, H], bf16)
            nc.scalar.dma_start(w2sb[:], w2_in[:])
            b2sb = wp.tile([128, G * OUT], f32)
            nc.scalar.dma_start(b2sb[:], b2_in[:])
            idfsb = wp.tile([128, 128], f32)
            nc.scalar.dma_start(idfsb[:], idf_in[:])

            nt = rpc // 128
            outsb = outp.tile([128, nt, OUT], f32)

            for u in range(nt // G):
                xtg = xtp.tile([128, KC, G * 128], bf16, tag="xtg")
                nc.sync.dma_start(xtg[:], xt_v[:, :, u * G * 128 : (u + 1) * G * 128])

                S = statp.tile([128, G, 2], f32, tag="S")
                OB = statp.tile([128, G, OUT], f32, tag="OB")
                pszs = []
                for q in range(G):
                    rs = q * 128
                    pszg = pszp.tile([128, H + 1], f32, tag="pszg")
                    pszs.append(pszg)
                    psg = psgp.tile([128, 128], f32, tag="psg")
                    for k in range(KC):
                        mm1 = nc.tensor.matmul(
                            pszg[:, 0 : H + 1], xtg[:, k, rs : rs + 128], w1sb[:, k, :],
                            start=(k == 0), stop=False,
                        )
                        mmg = nc.tensor.matmul(
                            psg[:],
                            xtg[:, k, rs : rs + 128], xtg[:, k, rs : rs + 128],
                            start=(k == 0), stop=(k == KC - 1),
                        )
                        mmg.ins.ldweights = False
                        add_dep_helper(mm1.ins, mmg.ins, False, "gram reuses stationary")
                    # -mu and sum(x^2) into the per-block stats tile
                    nc.vector.tensor_scalar_mul(S[:, q, 0:1], pszg[:, H : H + 1], -1.0 / D)
                    scr = scrp.tile([128, 128], f32, tag="scr")
                    nc.vector.scalar_tensor_tensor(
                        scr[:], idfsb[:], 1.0, psg[:],
                        ALU.mult, ALU.mult, accum_out=S[:, q, 1:2],
                    )

                # Batched stats: V = var+eps = SS/D - mu^2 + eps; Y = rsqrt(V).
                A1 = statp.tile([128, G], f32, tag="A1")
                nc.vector.tensor_scalar(A1[:], S[:, :, 1], 1.0 / D, EPS, ALU.mult, ALU.add)
                B = statp.tile([128, G], f32, tag="B")
                nc.vector.tensor_tensor(B[:], S[:, :, 0], S[:, :, 0], ALU.mult)
                V = statp.tile([128, G], f32, tag="V")
                nc.vector.tensor_tensor(V[:], A1[:], B[:], ALU.subtract)
                Y = statp.tile([128, G], f32, tag="Y")
                T = statp.tile([128, G], f32, tag="T")
                nc.vector.tensor_scalar(T[:].bitcast(i32), V[:].bitcast(i32), 1, None, ALU.logical_shift_right)
                nc.vector.tensor_scalar(Y[:].bitcast(i32), T[:].bitcast(i32), -1, MAGIC, ALU.mult, ALU.add)
                for _ in range(2):
                    nc.vector.tensor_tensor(T[:], V[:], Y[:], ALU.mult)
                    nc.vector.tensor_tensor(T[:], T[:], Y[:], ALU.mult)
                    nc.vector.tensor_scalar(T[:], T[:], -0.5, 1.5, ALU.mult, ALU.add)
                    nc.vector.tensor_tensor(Y[:], Y[:], T[:], ALU.mult)

                # BM cols 0:2G = interleaved (-mu, rhat) in bf16; xbar-flip to rows.
                BM = scrp.tile([128, 128], bf16, tag="BM")
                BMv = BM[:, 0 : 2 * G].rearrange("p (q s) -> p q s", s=2)
                nc.vector.tensor_copy(BMv[:, :, 0], S[:, :, 0])
                nc.vector.tensor_tensor(BMv[:, :, 1], V[:], Y[:], ALU.mult)
                BMT = scrp.tile([128, 128], bf16, tag="BMT")
                nc.sync.dma_start(BMT[:], BM[:], transpose=True)

                for q in range(G):
                    t = u * G + q
                    pszg = pszs[q]
                    nc.tensor.matmul(
                        pszg[:, 0 : H + 1], BMT[0 : 2 * G, :],
                        scsb[:, q, :], start=False, stop=True,
                    )
                    hb = hbp.tile([128, H], bf16, tag="hb")
                    nc.scalar.activation(
                        hb[:], pszg[:, 0:H], AF.Gelu, bias=0.0, scale=Y[:, q : q + 1]
                    )

                    scr2 = scrp.tile([128, H], f32, tag="scr2")
                    for c in range(OUT):
                        nc.vector.scalar_tensor_tensor(
                            scr2[:], hb[:], 1.0, w2sb[:, c, :],
                            ALU.mult, ALU.mult, accum_out=OB[:, q, c : c + 1],
                        )

                nc.vector.tensor_add(
                    outsb[:, u * G : (u + 1) * G, :].opt(),
                    OB[:].opt(), b2sb[:].rearrange("p (q c) -> p q c", c=OUT),
                )
                nc.scalar.dma_start(
                    y_v[:, u * G : (u + 1) * G, :], outsb[:, u * G : (u + 1) * G, :]
                )



    nc.finalize()
    return nc


def _get_nc():
    if "nc" not in _cache:
        _cache["nc"] = _build()
    return _cache["nc"]


def _prep_weights(ln_w, ln_b, W1, b1, W2, b2):
    W1p = ln_w[:, None] * W1                      # [1024, 256]
    s1 = W1p.sum(axis=0)                          # [256]
    c1 = ln_b @ W1 + b1                           # [256]
    w1aug = np.concatenate([W1p, np.ones((D, 1), np.float32)], axis=1)  # ones col -> rowsum
    sc = np.zeros((2 * G, G, H + 1), np.float32)
    for q in range(G):
        sc[2 * q, q, 0:H] = s1
        sc[2 * q + 1, q, 0:H] = c1
    return {
        "w1aug": _bf16(w1aug.reshape(KC, 128, H + 1).transpose(1, 0, 2)),
        "screp": _bf16(sc),
        "w2rep": _bf16(np.broadcast_to(W2.T, (128, OUT, H))),
        "b2g": np.broadcast_to(np.tile(b2, G), (128, G * OUT)).astype(np.float32).copy(),
        "identf": np.eye(128, dtype=np.float32),
    }


def _make_in_maps(embedding, ln_w, ln_b, W1, b1, W2, b2):
    embedding = np.asarray(embedding, dtype=np.float32)
    weights = _prep_weights(
        np.asarray(ln_w, dtype=np.float32), np.asarray(ln_b, dtype=np.float32),
        np.asarray(W1, dtype=np.float32), np.asarray(b1, dtype=np.float32),
        np.asarray(W2, dtype=np.float32), np.asarray(b2, dtype=np.float32),
    )
    xb = _bf16(embedding)                        # bf16 cast (rounding only)
    return [
        {"xt": np.ascontiguousarray(xb[c * RPC : (c + 1) * RPC].T), **weights}
        for c in range(N_CORES)
    ]


def kernel(embedding, ln_w, ln_b, W1, b1, W2, b2):
    from concourse.bass_utils import run_bass_kernel_spmd

    in_maps = _make_in_maps(embedding, ln_w, ln_b, W1, b1, W2, b2)
    nc = _get_nc()
    res = run_bass_kernel_spmd(nc, in_maps, core_ids=list(range(N_CORES)))
    out = np.concatenate([res.results[c]["y"] for c in range(N_CORES)], axis=0)
    return out.astype(np.float32)


# revision 23
# speedup vs baseline: 1.0819x; 1.0819x over previous
"""Trainium2 Bass kernel for nn_ClassificationHead: LayerNorm -> Linear(1024,256) -> GELU -> Linear(256,2).

Data-parallel over 8 NeuronCores: each core processes 8192 rows of the
65536-row batch; the tiny weights are replicated. The host supplies each
core's shard pre-transposed in bf16 (layout-only prep: [1024, 8192],
K-major as the tensor engine requires); all math runs on device.

Per-core pipeline, per 512-row block (4 tiles of 128 rows):
  1. One DMA loads the K-major block [128, 8, 512] bf16.
  2. Per tile, TensorE runs 8 accumulating matmuls against W1aug
     ([W1' | ones] -> PSUM cols 0:256 = x @ W1', col 256 = rowsum), plus a
     Gram matmul reusing the already-loaded stationary x-chunk
     (ldweights=False) into PSUM cols 257:385.
  3. DVE extracts -mu (from the rowsum col) and sum(x^2) (Gram diagonal via
     tensor_tensor_reduce against an identity); a batched Newton-rsqrt
     (bit-trick seed + 2 iterations) gives g = 1/sqrt(var+eps) and
     rhat = 1/g. A tiny [128,128] xbar-transpose DMA flips the per-row
     stats into rows.
  4. TensorE adds the rank-2 correction (-mu ox s1 + rhat ox c1), so after
     the GELU's per-partition scale g the PSUM holds exactly LN(x)@W1'+b1.
  5. ACT evaluates exact GELU with scale g -> bf16 h tile.
  6. TensorE transposes h (via identity), ACT evacuates PSUM->SBUF bf16,
     TensorE computes h @ W2; DVE adds b2 into a staging tile.
  7. One DMA writes the [8192, 2] fp32 result back.

Host-side weight folding (tiny, O(1MB)): W1' = ln_w[:,None]*W1,
s1 = colsum(W1'), c1 = ln_b@W1 + b1.
"""
import sys

sys.path.insert(0, "/opt/trn_rl_repo")
sys.path.insert(0, "/root/.axon_site")

import numpy as np
import ml_dtypes

N_CORES = 8
BATCH = 65536
D = 1024
H = 256
OUT = 2
RPC = BATCH // N_CORES  # rows per core
NT = RPC // 128         # 128-row tiles per core
KC = D // 128           # contraction chunks
G = 4                   # tiles per block (512 rows)
NB = NT // G            # blocks per core
EPS = 1e-5
MAGIC = 0x5F3759DF

_cache = {}


def _bf16(a):
    return np.asarray(a, dtype=ml_dtypes.bfloat16)


def _build(rpc=RPC):
    import concourse.bacc as bacc
    from concourse.tile_rust import add_dep_helper
    import concourse.mybir as mybir
    from concourse import tile

    f32 = mybir.dt.float32
    i32 = mybir.dt.int32
    bf16 = mybir.dt.bfloat16
    AF = mybir.ActivationFunctionType
    ALU = mybir.AluOpType

    nc = bacc.Bacc(None, target_bir_lowering=False, debug=False)

    xt_in = nc.dram_tensor("xt", [D, rpc], bf16, kind="ExternalInput")
    w1_in = nc.dram_tensor("w1aug", [128, KC, H + 1], bf16, kind="ExternalInput")
    sc_in = nc.dram_tensor("screp", [2 * G, G, H + 1], bf16, kind="ExternalInput")
    w2_in = nc.dram_tensor("w2rep", [128, OUT, H], bf16, kind="ExternalInput")
    b2_in = nc.dram_tensor("b2g", [128, G * OUT], f32, kind="ExternalInput")
    idf_in = nc.dram_tensor("identf", [128, 128], f32, kind="ExternalInput")
    y_out = nc.dram_tensor("y", [rpc, OUT], f32, kind="ExternalOutput")
    y_v = y_out.rearrange("(t p) c -> p t c", p=128)

    xt_v = xt_in.rearrange("(c p) r -> p c r", p=128)   # [128, KC, RPC]

    with tile.TileContext(nc) as tc:
        with (
            tc.tile_pool(name="wpool", bufs=1) as wp,
            tc.tile_pool(name="xtp", bufs=4) as xtp,
            tc.tile_pool(name="statp", bufs=2) as statp,
            tc.tile_pool(name="scrp", bufs=2) as scrp,
            tc.tile_pool(name="hbp", bufs=3) as hbp,
            tc.tile_pool(name="htp", bufs=3) as htp,
            tc.tile_pool(name="outp", bufs=1) as outp,
            tc.tile_pool(name="pszp", bufs=7, space="PSUM") as pszp,
            tc.tile_pool(name="psgp", bufs=1, space="PSUM") as psgp,
        ):
            w1sb = wp.tile([128, KC, H + 1], bf16)
            nc.sync.dma_start(w1sb[:], w1_in[:])
            scsb = wp.tile([2 * G, G, H + 1], bf16)
            nc.sync.dma_start(scsb[:], sc_in[:])
            w2sb = wp.tile([128, OUT, H], bf16)
            nc.sync.dma_start(w2sb[:], w2_in[:])
            b2sb = wp.tile([128, G * OUT], f32)
            nc.sync.dma_start(b2sb[:], b2_in[:])
            idfsb = wp.tile([128, 128], f32)
            nc.sync.dma_start(idfsb[:], idf_in[:])

            nt = rpc // 128
            outsb = outp.tile([128, nt, OUT], f32)

            for u in range(nt // G):
                xtg = xtp.tile([128, KC, G * 128], bf16, tag="xtg")
                nc.sync.dma_start(xtg[:], xt_v[:, :, u * G * 128 : (u + 1) * G * 128])

                S = statp.tile([128, G, 2], f32, tag="S")
                OB = statp.tile([128, G, OUT], f32, tag="OB")
                pszs = []
                for q in range(G):
                    rs = q * 128
                    pszg = pszp.tile([128, H + 1], f32, tag="pszg")
                    pszs.append(pszg)
                    psg = psgp.tile([128, 128], f32, tag="psg")
                    for k in range(KC):
                        mm1 = nc.tensor.matmul(
                            pszg[:, 0 : H + 1], xtg[:, k, rs : rs + 128], w1sb[:, k, :],
                            start=(k == 0), stop=False,
                        )
                        mmg = nc.tensor.matmul(
                            psg[:],
                            xtg[:, k, rs : rs + 128], xtg[:, k, rs : rs + 128],
                            start=(k == 0), stop=(k == KC - 1),
                        )
                        mmg.ins.ldweights = False
                        add_dep_helper(mm1.ins, mmg.ins, False, "gram reuses stationary")
                    # -mu and sum(x^2) into the per-block stats tile
                    nc.vector.tensor_scalar_mul(S[:, q, 0:1], pszg[:, H : H + 1], -1.0 / D)
                    scr = scrp.tile([128, 128], f32, tag="scr")
                    nc.vector.scalar_tensor_tensor(
                        scr[:], idfsb[:], 1.0, psg[:],
                        ALU.mult, ALU.mult, accum_out=S[:, q, 1:2],
                    )

                # Batched stats: V = var+eps = SS/D - mu^2 + eps; Y = rsqrt(V).
                A1 = statp.tile([128, G], f32, tag="A1")
                nc.vector.tensor_scalar(A1[:], S[:, :, 1], 1.0 / D, EPS, ALU.mult, ALU.add)
                B = statp.tile([128, G], f32, tag="B")
                nc.vector.tensor_tensor(B[:], S[:, :, 0], S[:, :, 0], ALU.mult)
                V = statp.tile([128, G], f32, tag="V")
                nc.vector.tensor_tensor(V[:], A1[:], B[:], ALU.subtract)
                Y = statp.tile([128, G], f32, tag="Y")
                T = statp.tile([128, G], f32, tag="T")
                nc.vector.tensor_scalar(T[:].bitcast(i32), V[:].bitcast(i32), 1, None, ALU.logical_shift_right)
                nc.vector.tensor_scalar(Y[:].bitcast(i32), T[:].bitcast(i32), -1, MAGIC, ALU.mult, ALU.add)
                for _ in range(2):
                    nc.vector.tensor_tensor(T[:], V[:], Y[:], ALU.mult)
                    nc.vector.tensor_tensor(T[:], T[:], Y[:], ALU.mult)
                    nc.vector.tensor_scalar(T[:], T[:], -0.5, 1.5, ALU.mult, ALU.add)
                    nc.vector.tensor_tensor(Y[:], Y[:], T[:], ALU.mult)

                # BM cols 0:2G = interleaved (-mu, rhat) in bf16; xbar-flip to rows.
                BM = scrp.tile([128, 128], bf16, tag="BM")
                BMv = BM[:, 0 : 2 * G].rearrange("p (q s) -> p q s", s=2)
                nc.vector.tensor_copy(BMv[:, :, 0], S[:, :, 0])
                nc.vector.tensor_tensor(BMv[:, :, 1], V[:], Y[:], ALU.mult)
                BMT = scrp.tile([128, 128], bf16, tag="BMT")
                nc.sync.dma_start(BMT[:], BM[:], transpose=True)

                for q in range(G):
                    t = u * G + q
                    pszg = pszs[q]
                    nc.tensor.matmul(
                        pszg[:, 0 : H + 1], BMT[0 : 2 * G, :],
                        scsb[:, q, :], start=False, stop=True,
                    )
                    hb = hbp.tile([128, H], bf16, tag="hb")
                    nc.scalar.activation(
                        hb[:], pszg[:, 0:H], AF.Gelu, bias=0.0, scale=Y[:, q : q + 1]
                    )

                    scr2 = scrp.tile([128, H], f32, tag="scr2")
                    for c in range(OUT):
                        nc.vector.scalar_tensor_tensor(
                            scr2[:], hb[:], 1.0, w2sb[:, c, :],
                            ALU.mult, ALU.mult, accum_out=OB[:, q, c : c + 1],
                        )

                nc.vector.tensor_add(
                    outsb[:, u * G : (u + 1) * G, :].opt(),
                    OB[:].opt(), b2sb[:].rearrange("p (q c) -> p q c", c=OUT),
                )
                nc.sync.dma_start(
                    y_v[:, u * G : (u + 1) * G, :], outsb[:, u * G : (u + 1) * G, :]
                )



    nc.finalize()
    return nc


def _get_nc():
    if "nc" not in _cache:
        _cache["nc"] = _build()
    return _cache["nc"]


def _prep_weights(ln_w, ln_b, W1, b1, W2, b2):
    W1p = ln_w[:, None] * W1                      # [1024, 256]
    s1 = W1p.sum(axis=0)                          # [256]
    c1 = ln_b @ W1 + b1                           # [256]
    w1aug = np.concatenate([W1p, np.ones((D, 1), np.float32)], axis=1)  # ones col -> rowsum
    sc = np.zeros((2 * G, G, H + 1), np.float32)
    for q in range(G):
        sc[2 * q, q, 0:H] = s1
        sc[2 * q + 1, q, 0:H] = c1
    return {
        "w1aug": _bf16(w1aug.reshape(KC, 128, H + 1).transpose(1, 0, 2)),
        "screp": _bf16(sc),
        "w2rep": _bf16(np.broadcast_to(W2.T, (128, OUT, H))),
        "b2g": np.broadcast_to(np.tile(b2, G), (128, G * OUT)).astype(np.float32).copy(),
        "identf": np.eye(128, dtype=np.float32),
    }


def _make_in_maps(embedding, ln_w, ln_b, W1, b1, W2, b2):
    embedding = np.asarray(embedding, dtype=np.float32)
    weights = _prep_weights(
        np.asarray(ln_w, dtype=np.float32), np.asarray(ln_b, dtype=np.float32),
        np.asarray(W1, dtype=np.float32), np.asarray(b1, dtype=np.float32),
        np.asarray(W2, dtype=np.float32), np.asarray(b2, dtype=np.float32),
    )
    xb = _bf16(embedding)                        # bf16 cast (rounding only)
    return [
        {"xt": np.ascontiguousarray(xb[c * RPC : (c + 1) * RPC].T), **weights}
        for c in range(N_CORES)
    ]


def kernel(embedding, ln_w, ln_b, W1, b1, W2, b2):
    from concourse.bass_utils import run_bass_kernel_spmd

    in_maps = _make_in_maps(embedding, ln_w, ln_b, W1, b1, W2, b2)
    nc = _get_nc()
    res = run_bass_kernel_spmd(nc, in_maps, core_ids=list(range(N_CORES)))
    out = np.concatenate([res.results[c]["y"] for c in range(N_CORES)], axis=0)
    return out.astype(np.float32)
